# revision 12
# baseline (speedup 1.0000x reference)
"""Multi-head attention (B=2, L=2048, D=1024, H=16) on 8 TRN2 NeuronCores.

Sharding: core c handles batch b = c//4 and head group g = c%4 (4 heads,
256 features). No inter-core communication; host sums the 4 per-head-group
output partials per batch and adds bo.

Per-core schedule (engine-balanced software pipeline):
  - warmup matmuls on a memset tile cover the ~9us framework/DMA startup and
    hold the PE HAM clock-gate at 2.4GHz; a tiny exp() preloads the ACT table
    and a dummy partition_broadcast preloads the GpSimd ucode library
  - input DMAs split across the Sync and GpSimd DGEs: xk on Sync (k proj is
    the scores gate), xq on GpSimd, xv halves on both
  - k projection (DMA-paced, d-outer), then q projection for query-block 0
    only; remaining q chunks and the v projection interleave with block-0
    scores so ScalarE starts exp'ing ~32us in
  - attention runs as 8 half-blocks (512 queries x head-pair). Per key tile m:
    the two heads' scores matmuls (K=64) issue back-to-back at PE row groups
    (0,0)/(64,0) so they stream concurrently in the array; one [128,1024] exp
    on ScalarE; two accumulating attn@V chain matmuls (ones-column emits the
    softmax denominators). The sc PSUM ring paces the PE to ScalarE's rate.
  - normalization: DVE reciprocal + GpSimd partition_broadcast + DVE multiply
    (no PE broadcast matmuls); output projection per query block with PSUM
    evacuation alternating between ScalarE and VectorE and output DMAs
    alternating between both DGE queues.
"""

import math
import sys

sys.path.insert(0, "/opt/trn_rl_repo")

import ml_dtypes
import numpy as np

import concourse.bass as bass
import concourse.mybir as mybir
import concourse.tile as tile
from concourse import bacc
from concourse.bass_utils import run_bass_kernel_spmd

B, L, D, H, DH = 2, 2048, 1024, 16, 64
NCORES = 8
HPC = 4                  # heads per core
FPC = HPC * DH           # 256 features per core
ND = D // 128            # 8 contraction tiles
NFT = FPC // 128         # 2 feature tiles for q/k/ctx
NM = L // 128            # 16 key tiles
VW = DH + 1              # 65 = head block width in v (64 feats + ones col)
VROW = HPC * VW          # 260
NQB = 4                  # 512-query blocks
SCALE = 1.0 / math.sqrt(DH)
CDT = mybir.dt.bfloat16
NP_CDT = ml_dtypes.bfloat16
F32 = mybir.dt.float32
EXP = mybir.ActivationFunctionType.Exp
OUT_NAME = "outT"
# half-blocks: (query block, head pair)
HB = [(qb, hp) for qb in range(NQB) for hp in range(2)]

_CACHE = {}


def build_nc():
    nc = bacc.Bacc(
        "TRN2",
        target_bir_lowering=False,
        debug=False,
        enable_asserts=False,
        num_devices=NCORES,
    )
    xqT_d = nc.dram_tensor("xqT", [D, L], CDT, kind="ExternalInput")
    xkT_d = nc.dram_tensor("xkT", [D, L], CDT, kind="ExternalInput")
    xvT_d = nc.dram_tensor("xvT", [D, L], CDT, kind="ExternalInput")
    wq_d = nc.dram_tensor("wqT", [D, FPC], CDT, kind="ExternalInput")
    wk_d = nc.dram_tensor("wkT", [D, FPC], CDT, kind="ExternalInput")
    wv_d = nc.dram_tensor("wvT", [D, VROW], CDT, kind="ExternalInput")
    wo_d = nc.dram_tensor("woT", [FPC, D], CDT, kind="ExternalInput")
    bq_d = nc.dram_tensor("bq2", [128, NFT], F32, kind="ExternalInput")
    bk_d = nc.dram_tensor("bk2", [128, NFT], F32, kind="ExternalInput")
    bvb_d = nc.dram_tensor("bvb", [128, VROW], F32, kind="ExternalInput")
    out_d = nc.dram_tensor(OUT_NAME, [D, L], CDT, kind="ExternalOutput")

    with tile.TileContext(nc) as tc:
        with tc.tile_pool(name="persist", bufs=1) as pp:
            qT = pp.tile([128, NFT, L], CDT)
            kT = pp.tile([128, NFT, L], CDT)
            vsb = pp.tile([128, NM, VROW], CDT)
            ctxT = pp.tile([128, NFT, L], CDT)
            wo_sb = pp.tile([128, NFT, D], CDT)
            bq_sb = pp.tile([128, NFT], F32)
            bk_sb = pp.tile([128, NFT], F32)
            bvb_sb = pp.tile([128, VROW], F32)
            warm = pp.tile([128, 512], CDT)
            actw = pp.tile([1, 16], F32)
            bcw_in = pp.tile([1, 16], F32)
            bcw = pp.tile([64, 16], F32)

            nc.vector.memset(warm[:], 0.25)
            nc.vector.memset(bcw_in[:], 1.0)
            # preload the exp ACT table set during the DMA-wait window
            nc.scalar.activation(actw[:], warm[0:1, 0:16], EXP, scale=SCALE)

            with tc.tile_pool(name="stageV", bufs=1) as sv:
                wv_sb = sv.tile([128, ND, VROW], CDT)
                xv_sb = sv.tile([128, ND, L], CDT)
                wv_r = wv_d.rearrange("(n p) f -> p n f", p=128)
                xv_r = xvT_d.rearrange("(n p) l -> p n l", p=128)

                with tc.tile_pool(name="stageQK", bufs=1) as sq:
                    wk_sb = sq.tile([128, ND, FPC], CDT)
                    xk_sb = sq.tile([128, ND, L], CDT)
                    wq_sb = sq.tile([128, ND, FPC], CDT)
                    xq_sb = sq.tile([128, ND, L], CDT)
                    xq_r = xqT_d.rearrange("(n p) l -> p n l", p=128)
                    xk_r = xkT_d.rearrange("(n p) l -> p n l", p=128)
                    wq_r = wq_d.rearrange("(n p) f -> p n f", p=128)
                    wk_r = wk_d.rearrange("(n p) f -> p n f", p=128)
                    # Both DGE queues share the 16 DMA engines, so arrival
                    # order == issue order by priority: xk (gates scores),
                    # then wq + xq's first query block (block-0 scores), then
                    # wv+xv, then the rest of xq, then wo. Even d-slices on
                    # Sync, odd on GpSimd.
                    nc.gpsimd.dma_start(bq_sb[:], bq_d[:])
                    nc.gpsimd.dma_start(bk_sb[:], bk_d[:])
                    nc.gpsimd.dma_start(bvb_sb[:], bvb_d[:])
                    for d in range(ND):
                        nc.sync.dma_start(wk_sb[:, d, :], wk_r[:, d, :])
                        eng = nc.sync if d % 2 == 0 else nc.gpsimd
                        eng.dma_start(xk_sb[:, d, :], xk_r[:, d, :])
                    for d in range(ND):
                        nc.gpsimd.dma_start(wq_sb[:, d, :], wq_r[:, d, :])
                    for d in range(ND):
                        eng = nc.sync if d % 2 == 0 else nc.gpsimd
                        eng.dma_start(xq_sb[:, d, 0:512], xq_r[:, d, 0:512])
                    # preload the GpSimd ucode library here (LOAD_LIB costs
                    # ~7us of GpSimd queue time; it must precede the first
                    # normalize but not delay the xk/xq descriptors)
                    nc.gpsimd.partition_broadcast(bcw[:], bcw_in[:], channels=64)
                    for d in range(ND):
                        eng = nc.sync if d % 2 == 0 else nc.gpsimd
                        eng.dma_start(wv_sb[:, d, :], wv_r[:, d, :])
                        eng2 = nc.gpsimd if d % 2 == 0 else nc.sync
                        eng2.dma_start(xv_sb[:, d, :], xv_r[:, d, :])
                    for d in range(ND):
                        eng = nc.sync if d % 2 == 0 else nc.gpsimd
                        eng.dma_start(xq_sb[:, d, 512:L], xq_r[:, d, 512:L])
                    nc.gpsimd.dma_start(
                        wo_sb[:], wo_d.rearrange("(n p) f -> p n f", p=128)
                    )

                    with tc.tile_pool(name="psW", bufs=1, space="PSUM") as psW:
                        wps = psW.tile([128, 512], F32)
                        # HAM warmup + cover DGE spin-up before inputs land
                        for i in range(22):
                            nc.tensor.matmul(
                                wps[:], warm[:, 0:128], warm[:],
                                start=True, stop=True, skip_group_check=True,
                            )
                        with tc.tile_pool(name="psA", bufs=4, space="PSUM") as psA:
                            # k projection: all 4 query... key chunks, d-outer
                            # so matmuls chase the DMA arrivals
                            for ft in range(NFT):
                                pss = [
                                    psA.tile([128, 512], F32, tag="pjk",
                                             name=f"pk_{ft}_{ch}")
                                    for ch in range(4)
                                ]
                                for d in range(ND):
                                    for ch in range(4):
                                        nc.tensor.matmul(
                                            pss[ch][:],
                                            wk_sb[:, d, ft * 128:(ft + 1) * 128],
                                            xk_sb[:, d, ch * 512:(ch + 1) * 512],
                                            start=(d == 0),
                                            stop=(d == ND - 1),
                                        )
                                for ch in range(4):
                                    nc.vector.tensor_scalar_add(
                                        kT[:, ft, ch * 512:(ch + 1) * 512],
                                        pss[ch][:],
                                        bk_sb[:, ft:ft + 1],
                                    )
                            # q projection, query-block 0 only (d-outer)
                            pss = [
                                psA.tile([128, 512], F32, tag="pjk",
                                         name=f"pq_{ft}_0")
                                for ft in range(NFT)
                            ]
                            for d in range(ND):
                                for ft in range(NFT):
                                    nc.tensor.matmul(
                                        pss[ft][:],
                                        wq_sb[:, d, ft * 128:(ft + 1) * 128],
                                        xq_sb[:, d, 0:512],
                                        start=(d == 0),
                                        stop=(d == ND - 1),
                                    )
                            for ft in range(NFT):
                                nc.vector.tensor_scalar_add(
                                    qT[:, ft, 0:512],
                                    pss[ft][:],
                                    bq_sb[:, ft:ft + 1],
                                )

                    # ---- attention pipeline ----
                    with (
                        tc.tile_pool(name="probs", bufs=20) as pb,
                        tc.tile_pool(name="smalls", bufs=2) as sm,
                        tc.tile_pool(name="psS", bufs=2, space="PSUM") as psS,
                        tc.tile_pool(name="psC", bufs=2, space="PSUM") as psC,
                    ):
                        probs = {}
                        chains = {}

                        def score_pair(si, m):
                            qb, hp = HB[si]
                            sc = psS.tile(
                                [128, 2, 512], F32, tag="sc", name=f"sc_{si}_{m}"
                            )
                            for hi in range(2):
                                po = hi * 64
                                nc.tensor.matmul(
                                    sc[:, hi, :],
                                    kT[po:po + 64, hp, m * 128:(m + 1) * 128],
                                    qT[po:po + 64, hp, qb * 512:(qb + 1) * 512],
                                    start=True,
                                    stop=True,
                                )
                            pr = pb.tile(
                                [128, 2, 512], CDT, tag="pr", name=f"pr_{si}_{m}"
                            )
                            nc.scalar.activation(pr[:], sc[:], EXP, scale=SCALE)
                            probs[(si, m)] = pr

                        def start_chains(si):
                            for hi in range(2):
                                chains[(si, hi)] = psC.tile(
                                    [VW, 512], F32, tag="ch", name=f"ch_{si}_{hi}"
                                )

                        def chain_m(si, m):
                            qb, hp = HB[si]
                            pr = probs[(si, m)]
                            for hi in range(2):
                                h = 2 * hp + hi
                                nc.tensor.matmul(
                                    chains[(si, hi)][:],
                                    vsb[:, m, h * VW:(h + 1) * VW],
                                    pr[:, hi, :],
                                    start=(m == 0),
                                    stop=(m == NM - 1),
                                )
                            del probs[(si, m)]

                        def normalize(si):
                            qb, hp = HB[si]
                            for hi in range(2):
                                ch = chains.pop((si, hi))
                                den = sm.tile(
                                    [1, 512], F32, tag="den", name=f"den_{si}_{hi}"
                                )
                                # reciprocal_approx_fast needs base partition 0
                                nc.vector.tensor_copy(den[:], ch[64:65, :])
                                rec = sm.tile(
                                    [1, 512], F32, tag="rec", name=f"rec_{si}_{hi}"
                                )
                                nc.vector.reciprocal_approx_fast(rec[:], den[:])
                                rbb = sm.tile(
                                    [64, 512], F32, tag="rbb", name=f"rbb_{si}_{hi}"
                                )
                                nc.gpsimd.partition_broadcast(
                                    rbb[:], rec[:], channels=64
                                )
                                po = hi * 64
                                nc.vector.tensor_mul(
                                    ctxT[po:po + 64, hp, qb * 512:(qb + 1) * 512],
                                    ch[0:64, :],
                                    rbb[:],
                                )

                        pending = []

                        def outproj_unit(qb, ft8, psX):
                            ops = psX.tile(
                                [128, 512], F32, tag="op", name=f"op_{qb}_{ft8}"
                            )
                            for d2 in range(NFT):
                                nc.tensor.matmul(
                                    ops[:],
                                    wo_sb[:, d2, ft8 * 128:(ft8 + 1) * 128],
                                    ctxT[:, d2, qb * 512:(qb + 1) * 512],
                                    start=(d2 == 0),
                                    stop=(d2 == NFT - 1),
                                )
                            st = sm.tile(
                                [128, 512], CDT, tag="ost", bufs=4,
                                name=f"st_{qb}_{ft8}",
                            )
                            # alternate evacuation engine and DMA queue so
                            # the MM->copy->DMA pipeline double-streams
                            if ft8 % 2 == 0:
                                nc.vector.tensor_copy(st[:], ops[:])
                            else:
                                nc.scalar.copy(st[:], ops[:])
                            dma_eng = nc.gpsimd if ft8 % 2 == 0 else nc.sync
                            dma_eng.dma_start(
                                out_d[
                                    ft8 * 128:(ft8 + 1) * 128,
                                    qb * 512:(qb + 1) * 512,
                                ],
                                st[:],
                            )

                        # deferred projection work units emitted as PE filler
                        # in the ScalarE-rate slack of the pipeline
                        def gen_qrest(chs, pool, tag):
                            # q projection chunks: per (ch, ft) an 8-matmul
                            # accumulation chain + bias add
                            for ch in chs:
                                for ft in range(NFT):
                                    ps = pool.tile(
                                        [128, 512], F32, tag=tag,
                                        name=f"pq_{ch}_{ft}",
                                    )
                                    for d in range(ND):
                                        yield lambda ps=ps, d=d, ft=ft, ch=ch: \
                                            nc.tensor.matmul(
                                                ps[:],
                                                wq_sb[:, d, ft * 128:(ft + 1) * 128],
                                                xq_sb[:, d, ch * 512:(ch + 1) * 512],
                                                start=(d == 0),
                                                stop=(d == ND - 1),
                                            )
                                    yield lambda ps=ps, ft=ft, ch=ch: \
                                        nc.vector.tensor_scalar_add(
                                            qT[:, ft, ch * 512:(ch + 1) * 512],
                                            ps[:],
                                            bq_sb[:, ft:ft + 1],
                                        )

                        def emit_filler(gen, n):
                            for _ in range(n):
                                op = next(gen, None)
                                if op is None:
                                    return
                                op()

                        with tc.tile_pool(name="psV", bufs=2, space="PSUM") as psV:
                            def gen_v():
                                for kt in range(NM):
                                    ps = psV.tile(
                                        [128, 512], F32, tag="pv",
                                        name=f"pv_{kt}",
                                    )
                                    for d in range(ND):
                                        yield lambda ps=ps, d=d, kt=kt: \
                                            nc.tensor.matmul(
                                                ps[:, 0:VROW],
                                                xv_sb[:, d, kt * 128:(kt + 1) * 128],
                                                wv_sb[:, d, :],
                                                start=(d == 0),
                                                stop=(d == ND - 1),
                                            )
                                    yield lambda ps=ps, kt=kt: \
                                        nc.vector.tensor_add(
                                            vsb[:, kt, :], ps[:, 0:VROW],
                                            bvb_sb[:],
                                        )

                            def chaingen(*gens):
                                for g in gens:
                                    yield from g

                            filler1 = chaingen(gen_v(), gen_qrest([1], psV, "pv"))

                            # stretch: block-0 scores paced by ScalarE; v
                            # filler starts once xv has landed (~m12)
                            for m in range(NM):
                                score_pair(0, m)
                                if m >= 12:
                                    emit_filler(filler1, 9)
                            # block 0 chains + block 1 scores + v/q filler
                            start_chains(0)
                            for m in range(NM):
                                chain_m(0, m)
                                score_pair(1, m)
                                emit_filler(filler1, 9)
                            emit_filler(filler1, 1000)
                            normalize(0)

                        with tc.tile_pool(name="psX", bufs=2, space="PSUM") as psX:
                            filler2 = gen_qrest([2, 3], psX, "op")
                            for i in range(2, len(HB) + 1):
                                prev = i - 1
                                start_chains(prev)
                                for m in range(NM):
                                    chain_m(prev, m)
                                    if i < len(HB):
                                        score_pair(i, m)
                                    # spread deferred q projection and the
                                    # previous block's output projection into
                                    # the ScalarE-rate slack
                                    if m % 2 == 0:
                                        if i <= 3:
                                            emit_filler(filler2, 4)
                                    elif pending:
                                        pending.pop(0)()
                                if i == 3:
                                    # q chunks 2/3 must precede block-2 scores
                                    emit_filler(filler2, 1000)
                                normalize(prev)
                                qb, hp = HB[prev]
                                if hp == 1:
                                    for ft8 in range(D // 128):
                                        pending.append(
                                            lambda qb=qb, ft8=ft8:
                                            outproj_unit(qb, ft8, psX)
                                        )
                            while pending:
                                pending.pop(0)()
    nc.compile()
    return nc


def make_in_maps(Q, K, V, Wq, bq, Wk, bk, Wv, bv, Wo, bo):
    Q = np.asarray(Q, np.float32)
    K = np.asarray(K, np.float32)
    V = np.asarray(V, np.float32)
    xqT = [np.ascontiguousarray(Q[b].T).astype(NP_CDT) for b in range(B)]
    xkT = [np.ascontiguousarray(K[b].T).astype(NP_CDT) for b in range(B)]
    xvT = [np.ascontiguousarray(V[b].T).astype(NP_CDT) for b in range(B)]
    in_maps = []
    for c in range(NCORES):
        b, g = divmod(c, HPC)
        fs = slice(g * FPC, (g + 1) * FPC)
        wqT = np.ascontiguousarray(np.asarray(Wq, np.float32)[fs, :].T).astype(NP_CDT)
        wkT = np.ascontiguousarray(np.asarray(Wk, np.float32)[fs, :].T).astype(NP_CDT)
        # v weights: per-head [64 cols | zero col], bias bcast carries the 1.0
        wv_blk = np.zeros((D, VROW), np.float32)
        bv_blk = np.zeros((VROW,), np.float32)
        wv_slc = np.asarray(Wv, np.float32)[fs, :].T  # [D, 256]
        bv_slc = np.asarray(bv, np.float32)[fs]
        for h in range(HPC):
            wv_blk[:, h * VW : h * VW + DH] = wv_slc[:, h * DH : (h + 1) * DH]
            bv_blk[h * VW : h * VW + DH] = bv_slc[h * DH : (h + 1) * DH]
            bv_blk[h * VW + DH] = 1.0
        woT = np.ascontiguousarray(np.asarray(Wo, np.float32)[:, fs].T).astype(NP_CDT)
        bq2 = np.ascontiguousarray(
            np.asarray(bq, np.float32)[fs].reshape(NFT, 128).T
        )
        bk2 = np.ascontiguousarray(
            np.asarray(bk, np.float32)[fs].reshape(NFT, 128).T
        )
        in_maps.append(
            {
                "xqT": xqT[b],
                "xkT": xkT[b],
                "xvT": xvT[b],
                "wqT": wqT,
                "wkT": wkT,
                "wvT": wv_blk.astype(NP_CDT),
                "woT": woT,
                "bq2": bq2,
                "bk2": bk2,
                "bvb": np.broadcast_to(bv_blk, (128, VROW)).copy(),
            }
        )
    return in_maps


def assemble(results, bo):
    out = np.zeros((B, L, D), np.float32)
    for c in range(NCORES):
        b = c // HPC
        out[b] += np.asarray(results[c][OUT_NAME], np.float32).T
    out += np.asarray(bo, np.float32)[None, None, :]
    return out


def kernel(Q, K, V, Wq, bq, Wk, bk, Wv, bv, Wo, bo):
    if "nc" not in _CACHE:
        _CACHE["nc"] = build_nc()
    nc = _CACHE["nc"]
    in_maps = make_in_maps(Q, K, V, Wq, bq, Wk, bk, Wv, bv, Wo, bo)
    res = run_bass_kernel_spmd(nc, in_maps, core_ids=list(range(NCORES)))
    return assemble(res.results, bo)


# revision 15
# speedup vs baseline: 1.0172x; 1.0172x over previous
"""Multi-head attention (B=2, L=2048, D=1024, H=16) on 8 TRN2 NeuronCores.

Sharding: core c handles batch b = c//4 and head group g = c%4 (4 heads,
256 features). No inter-core communication; host sums the 4 per-head-group
output partials per batch and adds bo.

Per-core schedule (engine-balanced software pipeline):
  - warmup matmuls on a memset tile cover the ~9us framework/DMA startup and
    hold the PE HAM clock-gate at 2.4GHz; a tiny exp() preloads the ACT table
    and a dummy partition_broadcast preloads the GpSimd ucode library
  - input DMAs split across the Sync and GpSimd DGEs: xk on Sync (k proj is
    the scores gate), xq on GpSimd, xv halves on both
  - k projection (DMA-paced, d-outer), then q projection for query-block 0
    only; remaining q chunks and the v projection interleave with block-0
    scores so ScalarE starts exp'ing ~32us in
  - attention runs as 8 half-blocks (512 queries x head-pair). Per key tile m:
    the two heads' scores matmuls (K=64) issue back-to-back at PE row groups
    (0,0)/(64,0) so they stream concurrently in the array; one [128,1024] exp
    on ScalarE; two accumulating attn@V chain matmuls (ones-column emits the
    softmax denominators). The sc PSUM ring paces the PE to ScalarE's rate.
  - normalization: DVE reciprocal + GpSimd partition_broadcast + DVE multiply
    (no PE broadcast matmuls); output projection per query block with PSUM
    evacuation alternating between ScalarE and VectorE and output DMAs
    alternating between both DGE queues.
"""

import math
import sys

sys.path.insert(0, "/opt/trn_rl_repo")

import ml_dtypes
import numpy as np

import concourse.bass as bass
import concourse.mybir as mybir
import concourse.tile as tile
from concourse import bacc
from concourse.bass_utils import run_bass_kernel_spmd

B, L, D, H, DH = 2, 2048, 1024, 16, 64
NCORES = 8
HPC = 4                  # heads per core
FPC = HPC * DH           # 256 features per core
ND = D // 128            # 8 contraction tiles
NFT = FPC // 128         # 2 feature tiles for q/k/ctx
NM = L // 128            # 16 key tiles
VW = DH + 1              # 65 = head block width in v (64 feats + ones col)
VROW = HPC * VW          # 260
NQB = 4                  # 512-query blocks
SCALE = 1.0 / math.sqrt(DH)
CDT = mybir.dt.bfloat16
NP_CDT = ml_dtypes.bfloat16
F32 = mybir.dt.float32
EXP = mybir.ActivationFunctionType.Exp
OUT_NAME = "outT"
# half-blocks: (query block, head pair)
HB = [(qb, hp) for qb in range(NQB) for hp in range(2)]

_CACHE = {}


def build_nc():
    nc = bacc.Bacc(
        "TRN2",
        target_bir_lowering=False,
        debug=False,
        enable_asserts=False,
        num_devices=NCORES,
    )
    # weights and xq are host-arranged partition-major so each loads with
    # one large-descriptor DMA (512B descriptors measured ~20us for 0.5MB)
    xqT_d = nc.dram_tensor("xqT", [128, NQB, ND, 512], CDT, kind="ExternalInput")
    xkT_d = nc.dram_tensor("xkT", [D, L], CDT, kind="ExternalInput")
    xvT_d = nc.dram_tensor("xvT", [D, L], CDT, kind="ExternalInput")
    wq_d = nc.dram_tensor("wqT", [128, ND, FPC], CDT, kind="ExternalInput")
    wk_d = nc.dram_tensor("wkT", [128, ND, FPC], CDT, kind="ExternalInput")
    wv_d = nc.dram_tensor("wvT", [128, ND, VROW], CDT, kind="ExternalInput")
    wo_d = nc.dram_tensor("woT", [128, NFT, D], CDT, kind="ExternalInput")
    bq_d = nc.dram_tensor("bq2", [128, NFT], F32, kind="ExternalInput")
    bk_d = nc.dram_tensor("bk2", [128, NFT], F32, kind="ExternalInput")
    bvb_d = nc.dram_tensor("bvb", [128, VROW], F32, kind="ExternalInput")
    out_d = nc.dram_tensor(OUT_NAME, [D, L], CDT, kind="ExternalOutput")

    with tile.TileContext(nc) as tc:
        with tc.tile_pool(name="persist", bufs=1) as pp:
            qT = pp.tile([128, NFT, L], CDT)
            kT = pp.tile([128, NFT, L], CDT)
            vsb = pp.tile([128, NM, VROW], CDT)
            ctxT = pp.tile([128, NFT, L], CDT)
            wo_sb = pp.tile([128, NFT, D], CDT)
            bq_sb = pp.tile([128, NFT], F32)
            bk_sb = pp.tile([128, NFT], F32)
            bvb_sb = pp.tile([128, VROW], F32)
            warm = pp.tile([128, 512], CDT)
            actw = pp.tile([1, 16], F32)
            bcw_in = pp.tile([1, 16], F32)
            bcw = pp.tile([64, 16], F32)

            nc.vector.memset(warm[:], 0.25)
            nc.vector.memset(bcw_in[:], 1.0)
            # preload the exp ACT table set during the DMA-wait window
            nc.scalar.activation(actw[:], warm[0:1, 0:16], EXP, scale=SCALE)

            with tc.tile_pool(name="stageV", bufs=1) as sv:
                wv_sb = sv.tile([128, ND, VROW], CDT)
                xv_sb = sv.tile([128, ND, L], CDT)
                xv_r = xvT_d.rearrange("(n p) l -> p n l", p=128)

                with tc.tile_pool(name="stageQK", bufs=1) as sq:
                    wk_sb = sq.tile([128, ND, FPC], CDT)
                    xk_sb = sq.tile([128, ND, L], CDT)
                    wq_sb = sq.tile([128, ND, FPC], CDT)
                    xq_sb = sq.tile([128, NQB, ND, 512], CDT)
                    xk_r = xkT_d.rearrange("(n p) l -> p n l", p=128)
                    # Both DGE queues share the 16 DMA engines, so arrival
                    # order == issue order by priority: xk (gates scores),
                    # then wq + xq's first query block (block-0 scores), then
                    # wv+xv, then the rest of xq, then wo. Even d-slices on
                    # Sync, odd on GpSimd.
                    nc.gpsimd.dma_start(bq_sb[:], bq_d[:])
                    nc.gpsimd.dma_start(bk_sb[:], bk_d[:])
                    nc.gpsimd.dma_start(bvb_sb[:], bvb_d[:])
                    nc.sync.dma_start(wk_sb[:], wk_d[:])
                    nc.gpsimd.dma_start(wq_sb[:], wq_d[:])
                    for d in range(ND):
                        eng = nc.sync if d % 2 == 0 else nc.gpsimd
                        eng.dma_start(xk_sb[:, d, :], xk_r[:, d, :])
                    # q inputs for query block 0 (gates the first scores)
                    nc.sync.dma_start(xq_sb[:, 0, 0:4, :], xqT_d[:, 0, 0:4, :])
                    nc.gpsimd.dma_start(xq_sb[:, 0, 4:8, :], xqT_d[:, 0, 4:8, :])
                    # preload the GpSimd ucode library here (LOAD_LIB costs
                    # ~7us of GpSimd queue time; it must precede the first
                    # normalize but not delay the xk/xq descriptors)
                    nc.gpsimd.partition_broadcast(bcw[:], bcw_in[:], channels=64)
                    nc.sync.dma_start(wv_sb[:], wv_d[:])
                    for d in range(ND):
                        eng = nc.sync if d % 2 == 0 else nc.gpsimd
                        eng.dma_start(xv_sb[:, d, :], xv_r[:, d, :])
                    for ch in range(1, NQB):
                        eng = nc.sync if ch % 2 == 0 else nc.gpsimd
                        eng.dma_start(xq_sb[:, ch, :, :], xqT_d[:, ch, :, :])
                    nc.gpsimd.dma_start(wo_sb[:], wo_d[:])

                    with tc.tile_pool(name="psW", bufs=1, space="PSUM") as psW:
                        wps = psW.tile([128, 512], F32)
                        # HAM warmup + cover DGE spin-up before inputs land
                        for i in range(22):
                            nc.tensor.matmul(
                                wps[:], warm[:, 0:128], warm[:],
                                start=True, stop=True, skip_group_check=True,
                            )
                        with tc.tile_pool(name="psA", bufs=4, space="PSUM") as psA:
                            # k projection: all 4 query... key chunks, d-outer
                            # so matmuls chase the DMA arrivals
                            for ft in range(NFT):
                                pss = [
                                    psA.tile([128, 512], F32, tag="pjk",
                                             name=f"pk_{ft}_{ch}")
                                    for ch in range(4)
                                ]
                                for d in range(ND):
                                    for ch in range(4):
                                        nc.tensor.matmul(
                                            pss[ch][:],
                                            wk_sb[:, d, ft * 128:(ft + 1) * 128],
                                            xk_sb[:, d, ch * 512:(ch + 1) * 512],
                                            start=(d == 0),
                                            stop=(d == ND - 1),
                                        )
                                for ch in range(4):
                                    nc.vector.tensor_scalar_add(
                                        kT[:, ft, ch * 512:(ch + 1) * 512],
                                        pss[ch][:],
                                        bk_sb[:, ft:ft + 1],
                                    )
                            # q projection, query-block 0 only (d-outer)
                            pss = [
                                psA.tile([128, 512], F32, tag="pjk",
                                         name=f"pq_{ft}_0")
                                for ft in range(NFT)
                            ]
                            for d in range(ND):
                                for ft in range(NFT):
                                    nc.tensor.matmul(
                                        pss[ft][:],
                                        wq_sb[:, d, ft * 128:(ft + 1) * 128],
                                        xq_sb[:, 0, d, :],
                                        start=(d == 0),
                                        stop=(d == ND - 1),
                                    )
                            for ft in range(NFT):
                                nc.vector.tensor_scalar_add(
                                    qT[:, ft, 0:512],
                                    pss[ft][:],
                                    bq_sb[:, ft:ft + 1],
                                )

                    # ---- attention pipeline ----
                    with (
                        tc.tile_pool(name="probs", bufs=20) as pb,
                        tc.tile_pool(name="smalls", bufs=2) as sm,
                        tc.tile_pool(name="psS", bufs=2, space="PSUM") as psS,
                        tc.tile_pool(name="psC", bufs=2, space="PSUM") as psC,
                    ):
                        probs = {}
                        chains = {}

                        def score_pair(si, m):
                            qb, hp = HB[si]
                            sc = psS.tile(
                                [128, 2, 512], F32, tag="sc", name=f"sc_{si}_{m}"
                            )
                            for hi in range(2):
                                po = hi * 64
                                nc.tensor.matmul(
                                    sc[:, hi, :],
                                    kT[po:po + 64, hp, m * 128:(m + 1) * 128],
                                    qT[po:po + 64, hp, qb * 512:(qb + 1) * 512],
                                    start=True,
                                    stop=True,
                                )
                            pr = pb.tile(
                                [128, 2, 512], CDT, tag="pr", name=f"pr_{si}_{m}"
                            )
                            nc.scalar.activation(pr[:], sc[:], EXP, scale=SCALE)
                            probs[(si, m)] = pr

                        def start_chains(si):
                            for hi in range(2):
                                chains[(si, hi)] = psC.tile(
                                    [VW, 512], F32, tag="ch", name=f"ch_{si}_{hi}"
                                )

                        def chain_m(si, m):
                            qb, hp = HB[si]
                            pr = probs[(si, m)]
                            for hi in range(2):
                                h = 2 * hp + hi
                                nc.tensor.matmul(
                                    chains[(si, hi)][:],
                                    vsb[:, m, h * VW:(h + 1) * VW],
                                    pr[:, hi, :],
                                    start=(m == 0),
                                    stop=(m == NM - 1),
                                )
                            del probs[(si, m)]

                        def normalize(si):
                            qb, hp = HB[si]
                            for hi in range(2):
                                ch = chains.pop((si, hi))
                                den = sm.tile(
                                    [1, 512], F32, tag="den", name=f"den_{si}_{hi}"
                                )
                                # reciprocal_approx_fast needs base partition 0
                                nc.vector.tensor_copy(den[:], ch[64:65, :])
                                rec = sm.tile(
                                    [1, 512], F32, tag="rec", name=f"rec_{si}_{hi}"
                                )
                                nc.vector.reciprocal_approx_fast(rec[:], den[:])
                                rbb = sm.tile(
                                    [64, 512], F32, tag="rbb", name=f"rbb_{si}_{hi}"
                                )
                                nc.gpsimd.partition_broadcast(
                                    rbb[:], rec[:], channels=64
                                )
                                po = hi * 64
                                nc.vector.tensor_mul(
                                    ctxT[po:po + 64, hp, qb * 512:(qb + 1) * 512],
                                    ch[0:64, :],
                                    rbb[:],
                                )

                        pending = []

                        def outproj_unit(qb, ft8, psX):
                            ops = psX.tile(
                                [128, 512], F32, tag="op", name=f"op_{qb}_{ft8}"
                            )
                            for d2 in range(NFT):
                                nc.tensor.matmul(
                                    ops[:],
                                    wo_sb[:, d2, ft8 * 128:(ft8 + 1) * 128],
                                    ctxT[:, d2, qb * 512:(qb + 1) * 512],
                                    start=(d2 == 0),
                                    stop=(d2 == NFT - 1),
                                )
                            st = sm.tile(
                                [128, 512], CDT, tag="ost", bufs=4,
                                name=f"st_{qb}_{ft8}",
                            )
                            # alternate evacuation engine and DMA queue so
                            # the MM->copy->DMA pipeline double-streams
                            if ft8 % 2 == 0:
                                nc.vector.tensor_copy(st[:], ops[:])
                            else:
                                nc.scalar.copy(st[:], ops[:])
                            dma_eng = nc.gpsimd if ft8 % 2 == 0 else nc.sync
                            dma_eng.dma_start(
                                out_d[
                                    ft8 * 128:(ft8 + 1) * 128,
                                    qb * 512:(qb + 1) * 512,
                                ],
                                st[:],
                            )

                        # deferred projection work units emitted as PE filler
                        # in the ScalarE-rate slack of the pipeline
                        def gen_qrest(chs, pool, tag):
                            # q projection chunks: per (ch, ft) an 8-matmul
                            # accumulation chain + bias add
                            for ch in chs:
                                for ft in range(NFT):
                                    ps = pool.tile(
                                        [128, 512], F32, tag=tag,
                                        name=f"pq_{ch}_{ft}",
                                    )
                                    for d in range(ND):
                                        yield lambda ps=ps, d=d, ft=ft, ch=ch: \
                                            nc.tensor.matmul(
                                                ps[:],
                                                wq_sb[:, d, ft * 128:(ft + 1) * 128],
                                                xq_sb[:, ch, d, :],
                                                start=(d == 0),
                                                stop=(d == ND - 1),
                                            )
                                    yield lambda ps=ps, ft=ft, ch=ch: \
                                        nc.vector.tensor_scalar_add(
                                            qT[:, ft, ch * 512:(ch + 1) * 512],
                                            ps[:],
                                            bq_sb[:, ft:ft + 1],
                                        )

                        def emit_filler(gen, n):
                            for _ in range(n):
                                op = next(gen, None)
                                if op is None:
                                    return
                                op()

                        with tc.tile_pool(name="psV", bufs=2, space="PSUM") as psV:
                            def gen_v():
                                for kt in range(NM):
                                    ps = psV.tile(
                                        [128, 512], F32, tag="pv",
                                        name=f"pv_{kt}",
                                    )
                                    for d in range(ND):
                                        yield lambda ps=ps, d=d, kt=kt: \
                                            nc.tensor.matmul(
                                                ps[:, 0:VROW],
                                                xv_sb[:, d, kt * 128:(kt + 1) * 128],
                                                wv_sb[:, d, :],
                                                start=(d == 0),
                                                stop=(d == ND - 1),
                                            )
                                    yield lambda ps=ps, kt=kt: \
                                        nc.vector.tensor_add(
                                            vsb[:, kt, :], ps[:, 0:VROW],
                                            bvb_sb[:],
                                        )

                            def chaingen(*gens):
                                for g in gens:
                                    yield from g

                            filler1 = chaingen(gen_v(), gen_qrest([1], psV, "pv"))

                            # stretch: block-0 scores paced by ScalarE; v
                            # filler starts once xv has landed (~m12)
                            for m in range(NM):
                                score_pair(0, m)
                                if m >= 12:
                                    emit_filler(filler1, 9)
                            # block 0 chains + block 1 scores + v/q filler
                            start_chains(0)
                            for m in range(NM):
                                chain_m(0, m)
                                score_pair(1, m)
                                emit_filler(filler1, 9)
                            emit_filler(filler1, 1000)
                            normalize(0)

                        with tc.tile_pool(name="psX", bufs=2, space="PSUM") as psX:
                            filler2 = gen_qrest([2, 3], psX, "op")
                            for i in range(2, len(HB) + 1):
                                prev = i - 1
                                start_chains(prev)
                                for m in range(NM):
                                    chain_m(prev, m)
                                    if i < len(HB):
                                        score_pair(i, m)
                                    # spread deferred q projection and the
                                    # previous block's output projection into
                                    # the ScalarE-rate slack
                                    if m % 2 == 0:
                                        if i <= 3:
                                            emit_filler(filler2, 4)
                                    elif pending:
                                        pending.pop(0)()
                                if i == 3:
                                    # q chunks 2/3 must precede block-2 scores
                                    emit_filler(filler2, 1000)
                                normalize(prev)
                                qb, hp = HB[prev]
                                if hp == 1:
                                    for ft8 in range(D // 128):
                                        pending.append(
                                            lambda qb=qb, ft8=ft8:
                                            outproj_unit(qb, ft8, psX)
                                        )
                            while pending:
                                pending.pop(0)()
    nc.compile()
    return nc


def _pmajor(a, n):
    """[n*128, F...] -> [128, n, F...] partition-major contiguous."""
    a = np.asarray(a)
    return np.ascontiguousarray(
        a.reshape(n, 128, *a.shape[1:]).swapaxes(0, 1)
    )


def make_in_maps(Q, K, V, Wq, bq, Wk, bk, Wv, bv, Wo, bo):
    Q = np.asarray(Q, np.float32)
    K = np.asarray(K, np.float32)
    V = np.asarray(V, np.float32)
    # xq host-arranged [128, qblock, d, 512]
    xqT = [
        np.ascontiguousarray(
            Q[b].T.reshape(ND, 128, NQB, 512).transpose(1, 2, 0, 3)
        ).astype(NP_CDT)
        for b in range(B)
    ]
    xkT = [np.ascontiguousarray(K[b].T).astype(NP_CDT) for b in range(B)]
    xvT = [np.ascontiguousarray(V[b].T).astype(NP_CDT) for b in range(B)]
    in_maps = []
    for c in range(NCORES):
        b, g = divmod(c, HPC)
        fs = slice(g * FPC, (g + 1) * FPC)
        wqT = _pmajor(np.asarray(Wq, np.float32)[fs, :].T, ND).astype(NP_CDT)
        wkT = _pmajor(np.asarray(Wk, np.float32)[fs, :].T, ND).astype(NP_CDT)
        # v weights: per-head [64 cols | zero col], bias bcast carries the 1.0
        wv_blk = np.zeros((D, VROW), np.float32)
        bv_blk = np.zeros((VROW,), np.float32)
        wv_slc = np.asarray(Wv, np.float32)[fs, :].T  # [D, 256]
        bv_slc = np.asarray(bv, np.float32)[fs]
        for h in range(HPC):
            wv_blk[:, h * VW : h * VW + DH] = wv_slc[:, h * DH : (h + 1) * DH]
            bv_blk[h * VW : h * VW + DH] = bv_slc[h * DH : (h + 1) * DH]
            bv_blk[h * VW + DH] = 1.0
        woT = _pmajor(np.asarray(Wo, np.float32)[:, fs].T, NFT).astype(NP_CDT)
        bq2 = np.ascontiguousarray(
            np.asarray(bq, np.float32)[fs].reshape(NFT, 128).T
        )
        bk2 = np.ascontiguousarray(
            np.asarray(bk, np.float32)[fs].reshape(NFT, 128).T
        )
        in_maps.append(
            {
                "xqT": xqT[b],
                "xkT": xkT[b],
                "xvT": xvT[b],
                "wqT": wqT,
                "wkT": wkT,
                "wvT": _pmajor(wv_blk, ND).astype(NP_CDT),
                "woT": woT,
                "bq2": bq2,
                "bk2": bk2,
                "bvb": np.broadcast_to(bv_blk, (128, VROW)).copy(),
            }
        )
    return in_maps


def assemble(results, bo):
    out = np.zeros((B, L, D), np.float32)
    for c in range(NCORES):
        b = c // HPC
        out[b] += np.asarray(results[c][OUT_NAME], np.float32).T
    out += np.asarray(bo, np.float32)[None, None, :]
    return out


def kernel(Q, K, V, Wq, bq, Wk, bk, Wv, bv, Wo, bo):
    if "nc" not in _CACHE:
        _CACHE["nc"] = build_nc()
    nc = _CACHE["nc"]
    in_maps = make_in_maps(Q, K, V, Wq, bq, Wk, bk, Wv, bv, Wo, bo)
    res = run_bass_kernel_spmd(nc, in_maps, core_ids=list(range(NCORES)))
    return assemble(res.results, bo)
